# revision 8
# baseline (speedup 1.0000x reference)
"""AnomalyAttention Trainium2 kernel (8 NeuronCores, SPMD).

Computes the Anomaly-Transformer attention block:
    scores = Q K^T (causal), series = softmax(scores / sqrt(E))
    prior[l,s] = N(|l-s|; 0, sig_l)  (row-wise Gaussian kernel)
    sig = transformed sigma, broadcast
    V_out = series @ V

Sharding: batch*heads (32 pairs) split 4-per-core across 8 cores; no
cross-core communication.  Host does layout shuffles only (transpose /
reshape / broadcast); all attention math runs on-device.
"""

import sys

if "/opt/trn_rl_repo" not in sys.path:
    sys.path.insert(0, "/opt/trn_rl_repo")

import math

import numpy as np

B, L, H, E = 4, 2048, 8, 64
NCORES = 8
PAIRS = B * H          # 32 (b,h) pairs
PER = PAIRS // NCORES  # 4 pairs per core
PB = 128               # partition block (rows per tile)
CH = 512               # score chunk width
NB = L // PB           # 16 row blocks per pair
NCH = L // CH          # 4 chunks per row
SCALE = 1.0 / math.sqrt(E)
LN3 = math.log(3.0)
HALF_LN_2PI = 0.5 * math.log(2.0 * math.pi)
NEG = -1.0e30

USE_F32R = True        # fast fp32 matmul mode on the PE
WRITE_ZERO_TAIL = False  # rely on zero-initialized ExternalOutput buffers

_NC_CACHE = {}


def build_nc():
    """Build + compile the per-core Bass graph (identical on all 8 cores)."""
    if "nc" in _NC_CACHE:
        return _NC_CACHE["nc"]

    import concourse.bass as bass
    import concourse.mybir as mybir
    import concourse.tile as tile
    from concourse import bacc
    from concourse.bass import ts
    from concourse.masks import make_identity

    f32 = mybir.dt.float32
    f32r = mybir.dt.float32r
    i32 = mybir.dt.int32
    Act = mybir.ActivationFunctionType
    Alu = mybir.AluOpType

    nc = bacc.Bacc("TRN2", target_bir_lowering=False, debug=False,
                   num_devices=NCORES)

    qT = nc.dram_tensor("qt", [PER, E, L], f32, kind="ExternalInput").ap()
    kT = nc.dram_tensor("kt", [PER, E, L], f32, kind="ExternalInput").ap()
    vI = nc.dram_tensor("v", [PER, L, E], f32, kind="ExternalInput").ap()
    sgI = nc.dram_tensor("sg", [PER, PB, NB], f32, kind="ExternalInput").ap()
    o_series = nc.dram_tensor("o_series", [PER, L, L], f32,
                              kind="ExternalOutput").ap()
    o_prior = nc.dram_tensor("o_prior", [PER, L, L], f32,
                             kind="ExternalOutput").ap()
    o_v = nc.dram_tensor("o_v", [PER, L, E], f32, kind="ExternalOutput").ap()
    o_sig = nc.dram_tensor("o_sig", [PER, PB, NB], f32,
                           kind="ExternalOutput").ap()

    with tile.TileContext(nc) as tc:
        with (
            tc.tile_pool(name="const", bufs=1) as constp,
            tc.tile_pool(name="initmp", bufs=1) as initmp,
            tc.tile_pool(name="qk", bufs=2) as qkp,
            tc.tile_pool(name="vin", bufs=2) as vinp,
            tc.tile_pool(name="sigp", bufs=2) as sigp,
            tc.tile_pool(name="pwork", bufs=6) as pwork,
            tc.tile_pool(name="ptbp", bufs=3) as ptbp,
            tc.tile_pool(name="priorp", bufs=3) as priorp,
            tc.tile_pool(name="vtp", bufs=2) as vtp,
            tc.tile_pool(name="voutp", bufs=2) as voutp,
            tc.tile_pool(name="psA", bufs=2, space="PSUM") as psA,
            tc.tile_pool(name="psT", bufs=2, space="PSUM") as psT,
            tc.tile_pool(name="psV", bufs=1, space="PSUM") as psV,
            tc.tile_pool(name="psR", bufs=1, space="PSUM") as psR,
        ):
            # ---- static constants -------------------------------------
            # d2[p, j] = (2047 + p - j)^2  -> row-block l0 reads the window
            # j in [2047-l0, 4095-l0) so that d2 = (l0+p - s)^2.
            d2i = initmp.tile([PB, 2 * L], i32)
            nc.gpsimd.iota(d2i, pattern=[[-1, 2 * L]], base=L - 1,
                           channel_multiplier=1)
            d2f = initmp.tile([PB, 2 * L], f32)
            nc.vector.tensor_copy(d2f, d2i)
            d2 = constp.tile([PB, 2 * L], f32)
            nc.vector.tensor_tensor(d2, d2f, d2f, Alu.mult)

            # additive causal masks for the diagonal 512-chunk, by row-block
            # offset m = i % 4: keep (0) where f <= 128*m + p else NEG.
            maskA = []
            for m in range(4):
                mk = constp.tile([PB, CH], f32, tag=f"maskA{m}")
                nc.gpsimd.memset(mk, 0.0)
                nc.gpsimd.affine_select(
                    out=mk, in_=mk, compare_op=Alu.is_ge, fill=NEG,
                    base=PB * m, pattern=[[-1, CH]], channel_multiplier=1)
                maskA.append(mk)

            ident = constp.tile([PB, PB], f32)
            make_identity(nc, ident)

            bias3 = constp.tile([PB, 1], f32, tag="bias3")
            nc.vector.memset(bias3, LN3 * 1e-5)

            zero_tail = None
            if WRITE_ZERO_TAIL:
                zero_tail = constp.tile([PB, L], f32)
                nc.vector.memset(zero_tail, 0.0)

            # ---- per-pair ("head") loop --------------------------------
            for h in range(PER):
                mmf = f32r if USE_F32R else f32
                mmdma = nc.gpsimd.dma_start if USE_F32R else nc.sync.dma_start
                qt_sb = qkp.tile([E, L], mmf, tag="qt")
                mmdma(qt_sb, qT[h])
                kt_sb = qkp.tile([E, L], mmf, tag="kt")
                mmdma(kt_sb, kT[h])
                v_sb = vinp.tile([PB, NB * E], mmf, tag="v")
                mmdma(v_sb, vI[h].rearrange("(n p) e -> p n e", p=PB))

                # sigma transform: sig = 3^(sigmoid(5x) + 1e-5) - 1
                sg_raw = sigp.tile([PB, NB], f32, tag="sgraw")
                nc.sync.dma_start(sg_raw, sgI[h])
                t1 = sigp.tile([PB, NB], f32, tag="t1")
                nc.scalar.activation(t1, sg_raw, Act.Sigmoid, scale=5.0)
                sigval = sigp.tile([PB, NB], f32, tag="sigval")
                nc.scalar.activation(sigval, t1, Act.Exp, bias=bias3[:, 0:1],
                                     scale=LN3)
                nc.vector.tensor_scalar_add(sigval, sigval, -1.0)
                nc.sync.dma_start(o_sig[h], sigval)

                # a = -1/(2 sig^2)   lnc = -ln(sig) - ln(sqrt(2 pi))
                sq = sigp.tile([PB, NB], f32, tag="sq")
                nc.vector.tensor_tensor(sq, sigval, sigval, Alu.mult)
                rq = sigp.tile([PB, NB], f32, tag="rq")
                nc.vector.reciprocal(rq, sq)
                a_coef = sigp.tile([PB, NB], f32, tag="acoef")
                nc.vector.tensor_scalar_mul(a_coef, rq, -0.5)
                lg = sigp.tile([PB, NB], f32, tag="lg")
                nc.scalar.activation(lg, sigval, Act.Ln)
                lnc = sigp.tile([PB, NB], f32, tag="lnc")
                nc.vector.tensor_scalar(lnc, lg, -1.0, -HALF_LN_2PI,
                                        Alu.mult, Alu.add)

                rs_all = sigp.tile([PB, NB], f32, tag="rsall")
                rinv_all = sigp.tile([PB, NB], f32, tag="rinvall")
                rs_parts = sigp.tile([PB, 2 * NB], f32, tag="rsparts")

                p_tiles = [None] * NB

                for i in range(NB):
                    l0 = i * PB

                    # ---------- prior row-block: one fused ACT op ------
                    pr = priorp.tile([PB, L], f32, tag="prior")
                    nc.scalar.activation(
                        pr, d2[:, L - 1 - l0:2 * L - 1 - l0], Act.Exp,
                        bias=lnc[:, i:i + 1], scale=a_coef[:, i:i + 1])
                    nc.scalar.dma_start(o_prior[h, ts(i, PB), :], pr)

                    # ---------- series row-block (phase A) -------------
                    nch = i // 4 + 1          # 512-chunks of valid scores
                    w = nch * CH
                    p_sb = pwork.tile([PB, L], f32, tag="p")
                    p_tiles[i] = p_sb
                    # scores into PSUM, [128, 1024] tiles (2 chunks each)
                    nhalf = (nch + 1) // 2
                    for g in range(nhalf):
                        ps = psA.tile([PB, 2 * CH], f32, tag="psa")
                        cw = min(2 * CH, w - g * 2 * CH)
                        for cc in range(g * 2, min(g * 2 + 2, nch)):
                            nc.tensor.matmul(
                                ps[:, ts(cc - g * 2, CH)],
                                qt_sb[:, ts(i, PB)],
                                kt_sb[:, ts(cc, CH)],
                                start=True, stop=True)
                        if g == nhalf - 1:
                            # diagonal chunk lives at the tail: mask it
                            moff = (nch - 1) - g * 2
                            nc.vector.tensor_tensor(
                                ps[:, ts(moff, CH)], ps[:, ts(moff, CH)],
                                maskA[i % 4], Alu.add)
                        nc.scalar.activation(
                            p_sb[:, g * 2 * CH:g * 2 * CH + cw], ps[:, 0:cw],
                            Act.Exp, scale=SCALE,
                            accum_out=rs_parts[:, 2 * i + g:2 * i + g + 1])
                    # rowsum -> reciprocal
                    if nhalf == 1:
                        nc.vector.reciprocal(rinv_all[:, i:i + 1],
                                             rs_parts[:, 2 * i:2 * i + 1])
                    else:
                        nc.vector.reduce_sum(
                            rs_all[:, i:i + 1],
                            rs_parts[:, 2 * i:2 * i + nhalf],
                            mybir.AxisListType.X)
                        nc.vector.reciprocal(rinv_all[:, i:i + 1],
                                             rs_all[:, i:i + 1])
                    nc.vector.tensor_scalar_mul(p_sb[:, 0:w], p_sb[:, 0:w],
                                                rinv_all[:, i:i + 1])
                    nc.sync.dma_start(o_series[h, ts(i, PB), 0:w],
                                      p_sb[:, 0:w])
                    if WRITE_ZERO_TAIL and w < L:
                        nc.sync.dma_start(o_series[h, ts(i, PB), w:L],
                                          zero_tail[:, w:L])

                    # ---------- phase B (every 4 row-blocks) ------------
                    if i % 4 != 3:
                        continue
                    j = i // 4
                    vt_ps = psV.tile([E, 4 * PB], f32, tag="vt")
                    for t in range(4 * j + 4):
                        m0 = max(0, t - 4 * j)   # first valid l-block
                        # build P^T[s-chunk t, l = 512j..512j+512) from the
                        # 4 row-block P tiles via PE transpose
                        tr_ps = psT.tile([PB, 4 * PB], f32, tag="tr")
                        for m in range(m0, 4):
                            nc.tensor.transpose(
                                tr_ps[:, ts(m, PB)],
                                p_tiles[4 * j + m][:, ts(t, PB)],
                                ident)
                        ptb = ptbp.tile([PB, 4 * PB], mmf, tag="ptb")
                        nc.vector.tensor_copy(ptb[:, m0 * PB:],
                                              tr_ps[:, m0 * PB:])
                        # V^T[e, l-superblock] += v_chunk^T . P^T chunk
                        nc.tensor.matmul(
                            vt_ps[:, m0 * PB:],
                            v_sb[:, ts(t, E)],
                            ptb[:, m0 * PB:],
                            start=(t == 0), stop=(t == 4 * j + 3))
                    vt_sb = vtp.tile([E, 4 * PB], f32, tag="vtsb")
                    nc.vector.tensor_copy(vt_sb, vt_ps)
                    tr2_ps = psR.tile([PB, 4 * E], f32, tag="tr2")
                    for m in range(4):
                        nc.tensor.transpose(
                            tr2_ps[:, ts(m, E)],
                            vt_sb[:, ts(m, PB)],
                            ident[0:E, 0:E])
                    if j == 0:
                        vout_sb = voutp.tile([PB, NB * E], f32, tag="vout")
                    nc.vector.tensor_copy(vout_sb[:, ts(j, 4 * E)], tr2_ps)
                    if j == 3:
                        nc.sync.dma_start(
                            o_v[h].rearrange("(n p) e -> p n e", p=PB),
                            vout_sb)

    nc.compile()
    _NC_CACHE["nc"] = nc
    return nc


def _shard_inputs(queries, keys, values, sigma):
    q = np.asarray(queries, dtype=np.float32)
    k = np.asarray(keys, dtype=np.float32)
    v = np.asarray(values, dtype=np.float32)
    sg = np.asarray(sigma, dtype=np.float32)

    qT = np.ascontiguousarray(q.transpose(0, 2, 3, 1)).reshape(PAIRS, E, L)
    kT = np.ascontiguousarray(k.transpose(0, 2, 3, 1)).reshape(PAIRS, E, L)
    vA = np.ascontiguousarray(v.transpose(0, 2, 1, 3)).reshape(PAIRS, L, E)
    # sigma [B, L, H] -> [pairs, L] -> [pairs, 128, 16] with l = n*128 + p
    sgA = sg.transpose(0, 2, 1).reshape(PAIRS, NB, PB).transpose(0, 2, 1)
    sgA = np.ascontiguousarray(sgA)

    in_maps = []
    for c in range(NCORES):
        s = slice(c * PER, (c + 1) * PER)
        in_maps.append({
            "qt": np.ascontiguousarray(qT[s]),
            "kt": np.ascontiguousarray(kT[s]),
            "v": np.ascontiguousarray(vA[s]),
            "sg": np.ascontiguousarray(sgA[s]),
        })
    return in_maps


def _assemble_outputs(results):
    series = np.concatenate([r["o_series"] for r in results], axis=0)
    series = series.reshape(B, H, L, L)
    prior = np.concatenate([r["o_prior"] for r in results], axis=0)
    prior = prior.reshape(B, H, L, L)
    vout = np.concatenate([r["o_v"] for r in results], axis=0)
    vout = vout.reshape(B, H, L, E).transpose(0, 2, 1, 3)
    vout = np.ascontiguousarray(vout)
    sig_pn = np.concatenate([r["o_sig"] for r in results], axis=0)
    sig_bhl = sig_pn.transpose(0, 2, 1).reshape(B, H, L)
    sig_full = np.broadcast_to(sig_bhl[..., None], (B, H, L, L))
    return vout, series, prior, sig_full


def kernel(**inputs):
    from concourse.bass_utils import run_bass_kernel_spmd

    nc = build_nc()
    in_maps = _shard_inputs(inputs["queries"], inputs["keys"],
                            inputs["values"], inputs["sigma"])
    res = run_bass_kernel_spmd(nc, in_maps, core_ids=list(range(NCORES)),
                               trace=False)
    return _assemble_outputs(res.results)


# revision 14
# speedup vs baseline: 1299.3860x; 1299.3860x over previous
"""AnomalyAttention Trainium2 kernel (8 NeuronCores, SPMD).

Computes the Anomaly-Transformer attention block:
    scores = Q K^T (causal), series = softmax(scores / sqrt(E))
    prior[l,s] = N(|l-s|; 0, sig_l)  (row-wise Gaussian kernel)
    sig = transformed sigma, broadcast
    V_out = series @ V

Sharding: batch*heads (32 pairs) split 4-per-core across 8 cores; no
cross-core communication.  Host does layout shuffles only (transpose /
reshape / broadcast); all attention math runs on-device.

prior is band-limited in fp32: sig <= 2.0 so exp(-d^2/(2 sig^2))
underflows to exactly 0.0f for |d| >= 29; only a +-BW diagonal band is
computed/written, the rest of the (zero-initialized) output stays 0.
"""

import sys

if "/opt/trn_rl_repo" not in sys.path:
    sys.path.insert(0, "/opt/trn_rl_repo")

import math

import numpy as np

B, L, H, E = 4, 2048, 8, 64
NCORES = 8
PAIRS = B * H          # 32 (b,h) pairs
PER = PAIRS // NCORES  # 4 pairs per core
PB = 128               # partition block (rows per tile)
CH = 512               # score chunk width
NB = L // PB           # 16 row blocks per pair
NCH = L // CH          # 4 chunks per row
SCALE = 1.0 / math.sqrt(E)
LN3 = math.log(3.0)
HALF_LN_2PI = 0.5 * math.log(2.0 * math.pi)
NEG = -1.0e30
BW = 32  # prior band halfwidth: exp(-d^2/(2*sig^2)) == 0 in f32 for |d| >= 29

USE_F32R = True        # fast fp32 matmul mode on the PE
WRITE_ZERO_TAIL = False  # rely on zero-initialized ExternalOutput buffers

import os
NORM_ENGINE = os.environ.get("AK_NORM", "dve")   # gpsimd | dve | skip
LOAD_PATH = os.environ.get("AK_LOADS", "swdge")     # swdge | synccast
SKIP_B = os.environ.get("AK_SKIP_B", "0") == "1"
SKIP_PRIOR = os.environ.get("AK_SKIP_PRIOR", "0") == "1"
SKIP_SERIES_DMA = os.environ.get("AK_SKIP_SDMA", "0") == "1"

_NC_CACHE = {}


def build_nc(repeat=1):
    """Build + compile the per-core Bass graph (identical on all 8 cores).

    repeat>1 wraps the computation in a hardware loop (identical work
    each iteration) — used only for on-device timing measurement.
    """
    key = (repeat, NORM_ENGINE, LOAD_PATH, SKIP_B, SKIP_PRIOR,
           SKIP_SERIES_DMA, USE_F32R)
    if key in _NC_CACHE:
        return _NC_CACHE[key]

    import concourse.bass as bass
    import concourse.mybir as mybir
    import concourse.tile as tile
    from concourse import bacc
    from concourse.bass import ts
    from concourse.masks import make_identity

    f32 = mybir.dt.float32
    f32r = mybir.dt.float32r
    i32 = mybir.dt.int32
    Act = mybir.ActivationFunctionType
    Alu = mybir.AluOpType

    nc = bacc.Bacc("TRN2", target_bir_lowering=False, debug=False,
                   num_devices=NCORES)

    qT = nc.dram_tensor("qt", [PER, E, L], f32, kind="ExternalInput").ap()
    kT = nc.dram_tensor("kt", [PER, E, L], f32, kind="ExternalInput").ap()
    vI = nc.dram_tensor("v", [PER, L, E], f32, kind="ExternalInput").ap()
    sgI = nc.dram_tensor("sg", [PER, PB, NB], f32, kind="ExternalInput").ap()
    o_series = nc.dram_tensor("o_series", [PER, L, L], f32,
                              kind="ExternalOutput").ap()
    o_prior = nc.dram_tensor("o_prior", [PER, L, L], f32,
                             kind="ExternalOutput").ap()
    o_v = nc.dram_tensor("o_v", [PER, L, E], f32, kind="ExternalOutput").ap()
    o_sig = nc.dram_tensor("o_sig", [PER, PB, NB], f32,
                           kind="ExternalOutput").ap()

    with tile.TileContext(nc) as tc:
        with (
            tc.tile_pool(name="const", bufs=1) as constp,
            tc.tile_pool(name="initmp", bufs=1) as initmp,
            tc.tile_pool(name="qk", bufs=2) as qkp,
            tc.tile_pool(name="vin", bufs=2) as vinp,
            tc.tile_pool(name="sigp", bufs=2) as sigp,
            tc.tile_pool(name="pwork", bufs=6) as pwork,
            tc.tile_pool(name="ptbp", bufs=3) as ptbp,
            tc.tile_pool(name="priorp", bufs=3) as priorp,
            tc.tile_pool(name="vtp", bufs=2) as vtp,
            tc.tile_pool(name="voutp", bufs=2) as voutp,
            tc.tile_pool(name="psA", bufs=2, space="PSUM") as psA,
            tc.tile_pool(name="psT", bufs=2, space="PSUM") as psT,
            tc.tile_pool(name="psV", bufs=1, space="PSUM") as psV,
            tc.tile_pool(name="psR", bufs=1, space="PSUM") as psR,
        ):
            # ---- static constants -------------------------------------
            # d2[p, j] = (2047 + p - j)^2  -> row-block l0 reads the window
            # j in [2047-l0, 4095-l0) so that d2 = (l0+p - s)^2.
            d2i = initmp.tile([PB, 2 * L], i32)
            nc.gpsimd.iota(d2i, pattern=[[-1, 2 * L]], base=L - 1,
                           channel_multiplier=1)
            d2f = initmp.tile([PB, 2 * L], f32)
            nc.vector.tensor_copy(d2f, d2i)
            d2 = constp.tile([PB, 2 * L], f32)
            nc.vector.tensor_tensor(d2, d2f, d2f, Alu.mult)

            # additive causal masks for the diagonal 512-chunk, by row-block
            # offset m = i % 4: keep (0) where f <= 128*m + p else NEG.
            maskA = []
            for m in range(4):
                mk = constp.tile([PB, CH], f32, tag=f"maskA{m}")
                nc.gpsimd.memset(mk, 0.0)
                nc.gpsimd.affine_select(
                    out=mk, in_=mk, compare_op=Alu.is_ge, fill=NEG,
                    base=PB * m, pattern=[[-1, CH]], channel_multiplier=1)
                maskA.append(mk)

            ident = constp.tile([PB, PB], f32)
            make_identity(nc, ident)

            bias3 = constp.tile([PB, 1], f32, tag="bias3")
            nc.vector.memset(bias3, LN3 * 1e-5)

            mmf = f32r if USE_F32R else f32
            mmdma = nc.gpsimd.dma_start if USE_F32R else nc.sync.dma_start

            def body():
                # ---- sigma transforms, grouped per ACT function so
                # walrus emits few ACT table switches --------------------
                sig_t1, sig_val, sig_a, sig_lnc = {}, {}, {}, {}
                for h in range(PER):
                    sg_raw = sigp.tile([PB, NB], f32, tag=f"sgraw{h}", bufs=1)
                    nc.sync.dma_start(sg_raw, sgI[h])
                    t1 = sigp.tile([PB, NB], f32, tag=f"t1{h}", bufs=1)
                    nc.scalar.activation(t1, sg_raw, Act.Sigmoid, scale=5.0)
                    sig_t1[h] = t1
                for h in range(PER):
                    # sig = 3^(sigmoid(5x) + 1e-5) - 1
                    sigval = sigp.tile([PB, NB], f32, tag=f"sigval{h}",
                                       bufs=1)
                    nc.scalar.activation(sigval, sig_t1[h], Act.Exp,
                                         bias=bias3[:, 0:1], scale=LN3)
                    nc.vector.tensor_scalar_add(sigval, sigval, -1.0)
                    nc.sync.dma_start(o_sig[h], sigval)
                    sig_val[h] = sigval
                for h in range(PER):
                    sigval = sig_val[h]
                    sq = sigp.tile([PB, NB], f32, tag=f"sq{h}", bufs=1)
                    nc.vector.tensor_tensor(sq, sigval, sigval, Alu.mult)
                    rq = sigp.tile([PB, NB], f32, tag=f"rq{h}", bufs=1)
                    nc.vector.reciprocal(rq, sq)
                    a_coef = sigp.tile([PB, NB], f32, tag=f"acoef{h}", bufs=1)
                    nc.vector.tensor_scalar_mul(a_coef, rq, -0.5)
                    sig_a[h] = a_coef
                    lg = sigp.tile([PB, NB], f32, tag=f"lg{h}", bufs=1)
                    nc.scalar.activation(lg, sigval, Act.Ln)
                    lnc = sigp.tile([PB, NB], f32, tag=f"lnc{h}", bufs=1)
                    nc.vector.tensor_scalar(lnc, lg, -1.0, -HALF_LN_2PI,
                                            Alu.mult, Alu.add)
                    sig_lnc[h] = lnc

                # ---- per-pair ("head") loop ----------------------------
                for h in range(PER):
                    if LOAD_PATH == "swdge" or not USE_F32R:
                        qt_sb = qkp.tile([E, L], mmf, tag="qt")
                        mmdma(qt_sb, qT[h])
                        kt_sb = qkp.tile([E, L], mmf, tag="kt")
                        mmdma(kt_sb, kT[h])
                        v_sb = vinp.tile([PB, NB * E], mmf, tag="v")
                        mmdma(v_sb, vI[h].rearrange("(n p) e -> p n e",
                                                    p=PB))
                    else:
                        qt_f = qkp.tile([E, L], f32, tag="qtf")
                        nc.sync.dma_start(qt_f, qT[h])
                        qt_sb = qkp.tile([E, L], mmf, tag="qt")
                        nc.vector.tensor_copy(qt_sb, qt_f)
                        kt_f = qkp.tile([E, L], f32, tag="ktf")
                        nc.sync.dma_start(kt_f, kT[h])
                        kt_sb = qkp.tile([E, L], mmf, tag="kt")
                        nc.vector.tensor_copy(kt_sb, kt_f)
                        v_f = vinp.tile([PB, NB * E], f32, tag="vf")
                        nc.sync.dma_start(
                            v_f, vI[h].rearrange("(n p) e -> p n e", p=PB))
                        v_sb = vinp.tile([PB, NB * E], mmf, tag="v")
                        nc.vector.tensor_copy(v_sb, v_f)

                    a_coef, lnc = sig_a[h], sig_lnc[h]
                    rs_parts = sigp.tile([PB, 2 * NB], f32, tag="rsparts")
                    rs_all = sigp.tile([PB, NB], f32, tag="rsall")
                    rinv_all = sigp.tile([PB, NB], f32, tag="rinvall")

                    p_tiles = [None] * NB
                    vout_sb = None

                    for i in range(NB):
                        l0 = i * PB

                        # ---- prior row-block: banded -------------------
                        c0 = max(0, l0 - BW)
                        c1 = min(L, l0 + PB + BW)
                        W = c1 - c0
                        pr = priorp.tile([PB, PB + 2 * BW], f32, tag="prior")
                        if SKIP_PRIOR:
                            pr = None
                        if not SKIP_PRIOR:
                            nc.scalar.activation(
                            pr[:, 0:W],
                            d2[:, L - 1 - l0 + c0:L - 1 - l0 + c0 + W],
                                Act.Exp, bias=lnc[:, i:i + 1],
                                scale=a_coef[:, i:i + 1])
                            nc.scalar.dma_start(
                                o_prior[h, ts(i, PB), c0:c1], pr[:, 0:W])

                        # ---- series row-block (phase A) ----------------
                        nch = i // 4 + 1
                        w = nch * CH
                        p_sb = pwork.tile([PB, L], f32, tag="p")
                        p_tiles[i] = p_sb
                        nhalf = (nch + 1) // 2
                        for g in range(nhalf):
                            ps = psA.tile([PB, 2 * CH], f32, tag="psa")
                            cw = min(2 * CH, w - g * 2 * CH)
                            for cc in range(g * 2, min(g * 2 + 2, nch)):
                                nc.tensor.matmul(
                                    ps[:, ts(cc - g * 2, CH)],
                                    qt_sb[:, ts(i, PB)],
                                    kt_sb[:, ts(cc, CH)],
                                    start=True, stop=True)
                            if g == nhalf - 1:
                                # mask the diag chunk's maskable columns
                                moff = (nch - 1) - g * 2
                                mb0 = PB * (i % 4)
                                nc.vector.tensor_tensor(
                                    ps[:, moff * CH + mb0:(moff + 1) * CH],
                                    ps[:, moff * CH + mb0:(moff + 1) * CH],
                                    maskA[i % 4][:, mb0:], Alu.add)
                            nc.scalar.activation(
                                p_sb[:, g * 2 * CH:g * 2 * CH + cw],
                                ps[:, 0:cw], Act.Exp, scale=SCALE,
                                accum_out=rs_parts[:,
                                                   2 * i + g:2 * i + g + 1])
                        if nhalf == 1:
                            nc.vector.reciprocal(
                                rinv_all[:, i:i + 1],
                                rs_parts[:, 2 * i:2 * i + 1])
                        else:
                            nc.vector.reduce_sum(
                                rs_all[:, i:i + 1],
                                rs_parts[:, 2 * i:2 * i + nhalf],
                                mybir.AxisListType.X)
                            nc.vector.reciprocal(rinv_all[:, i:i + 1],
                                                 rs_all[:, i:i + 1])
                        if NORM_ENGINE == "gpsimd":
                            nc.gpsimd.tensor_scalar_mul(
                                p_sb[:, 0:w], p_sb[:, 0:w],
                                rinv_all[:, i:i + 1])
                        elif NORM_ENGINE == "dve":
                            nc.vector.tensor_scalar_mul(
                                p_sb[:, 0:w], p_sb[:, 0:w],
                                rinv_all[:, i:i + 1])
                        if not SKIP_SERIES_DMA:
                            nc.sync.dma_start(o_series[h, ts(i, PB), 0:w],
                                              p_sb[:, 0:w])

                        # ---- phase B every 4 row-blocks ----------------
                        if SKIP_B or i % 4 != 3:
                            continue
                        j = i // 4
                        vt_ps = psV.tile([E, 4 * PB], f32, tag="vt")
                        for t in range(4 * j + 4):
                            m0 = max(0, t - 4 * j)
                            tr_ps = psT.tile([PB, 4 * PB], f32, tag="tr")
                            for m in range(m0, 4):
                                nc.tensor.transpose(
                                    tr_ps[:, ts(m, PB)],
                                    p_tiles[4 * j + m][:, ts(t, PB)],
                                    ident)
                            ptb = ptbp.tile([PB, 4 * PB], mmf, tag="ptb")
                            nc.vector.tensor_copy(ptb[:, m0 * PB:],
                                                  tr_ps[:, m0 * PB:])
                            nc.tensor.matmul(
                                vt_ps[:, m0 * PB:],
                                v_sb[:, ts(t, E)],
                                ptb[:, m0 * PB:],
                                start=(t == 0), stop=(t == 4 * j + 3))
                        vt_sb = vtp.tile([E, 4 * PB], f32, tag="vtsb")
                        nc.vector.tensor_copy(vt_sb, vt_ps)
                        tr2_ps = psR.tile([PB, 4 * E], f32, tag="tr2")
                        for m in range(4):
                            nc.tensor.transpose(
                                tr2_ps[:, ts(m, E)],
                                vt_sb[:, ts(m, PB)],
                                ident[0:E, 0:E])
                        if j == 0:
                            vout_sb = voutp.tile([PB, NB * E], f32,
                                                 tag="vout")
                        nc.vector.tensor_copy(vout_sb[:, ts(j, 4 * E)],
                                              tr2_ps)
                        if j == 3:
                            nc.sync.dma_start(
                                o_v[h].rearrange("(n p) e -> p n e", p=PB),
                                vout_sb)

            if repeat > 1:
                with tc.For_i(0, repeat, 1):
                    body()
            else:
                body()

    nc.compile()
    _NC_CACHE[key] = nc
    return nc


def _shard_inputs(queries, keys, values, sigma):
    q = np.asarray(queries, dtype=np.float32)
    k = np.asarray(keys, dtype=np.float32)
    v = np.asarray(values, dtype=np.float32)
    sg = np.asarray(sigma, dtype=np.float32)

    qT = np.ascontiguousarray(q.transpose(0, 2, 3, 1)).reshape(PAIRS, E, L)
    kT = np.ascontiguousarray(k.transpose(0, 2, 3, 1)).reshape(PAIRS, E, L)
    vA = np.ascontiguousarray(v.transpose(0, 2, 1, 3)).reshape(PAIRS, L, E)
    # sigma [B, L, H] -> [pairs, L] -> [pairs, 128, 16] with l = n*128 + p
    sgA = sg.transpose(0, 2, 1).reshape(PAIRS, NB, PB).transpose(0, 2, 1)
    sgA = np.ascontiguousarray(sgA)

    in_maps = []
    for c in range(NCORES):
        s = slice(c * PER, (c + 1) * PER)
        in_maps.append({
            "qt": np.ascontiguousarray(qT[s]),
            "kt": np.ascontiguousarray(kT[s]),
            "v": np.ascontiguousarray(vA[s]),
            "sg": np.ascontiguousarray(sgA[s]),
        })
    return in_maps


def _assemble_outputs(results):
    series = np.concatenate([r["o_series"] for r in results], axis=0)
    series = series.reshape(B, H, L, L)
    prior = np.concatenate([r["o_prior"] for r in results], axis=0)
    prior = prior.reshape(B, H, L, L)
    vout = np.concatenate([r["o_v"] for r in results], axis=0)
    vout = vout.reshape(B, H, L, E).transpose(0, 2, 1, 3)
    vout = np.ascontiguousarray(vout)
    sig_pn = np.concatenate([r["o_sig"] for r in results], axis=0)
    sig_bhl = sig_pn.transpose(0, 2, 1).reshape(B, H, L)
    sig_full = np.broadcast_to(sig_bhl[..., None], (B, H, L, L))
    return vout, series, prior, sig_full


def kernel(**inputs):
    from concourse.bass_utils import run_bass_kernel_spmd

    nc = build_nc()
    in_maps = _shard_inputs(inputs["queries"], inputs["keys"],
                            inputs["values"], inputs["sigma"])
    res = run_bass_kernel_spmd(nc, in_maps, core_ids=list(range(NCORES)),
                               trace=False)
    return _assemble_outputs(res.results)


# revision 16
# speedup vs baseline: 1320.9722x; 1.0166x over previous
"""AnomalyAttention Trainium2 kernel (8 NeuronCores, SPMD).

Computes the Anomaly-Transformer attention block:
    scores = Q K^T (causal), series = softmax(scores / sqrt(E))
    prior[l,s] = N(|l-s|; 0, sig_l)  (row-wise Gaussian kernel)
    sig = transformed sigma, broadcast
    V_out = series @ V

Sharding: batch*heads (32 pairs) split 4-per-core across 8 cores; no
cross-core communication.  Host does layout shuffles only (transpose /
reshape / broadcast); all attention math runs on-device.

prior is band-limited in fp32: sig <= 2.0 so exp(-d^2/(2 sig^2))
underflows to exactly 0.0f for |d| >= 29; only a +-BW diagonal band is
computed/written, the rest of the (zero-initialized) output stays 0.
"""

import sys

if "/opt/trn_rl_repo" not in sys.path:
    sys.path.insert(0, "/opt/trn_rl_repo")

import math

import numpy as np

B, L, H, E = 4, 2048, 8, 64
NCORES = 8
PAIRS = B * H          # 32 (b,h) pairs
PER = PAIRS // NCORES  # 4 pairs per core
PB = 128               # partition block (rows per tile)
CH = 512               # score chunk width
NB = L // PB           # 16 row blocks per pair
NCH = L // CH          # 4 chunks per row
SCALE = 1.0 / math.sqrt(E)
LN3 = math.log(3.0)
HALF_LN_2PI = 0.5 * math.log(2.0 * math.pi)
NEG = -1.0e30
BW = 32  # prior band halfwidth: exp(-d^2/(2*sig^2)) == 0 in f32 for |d| >= 29

USE_F32R = True        # fast fp32 matmul mode on the PE
WRITE_ZERO_TAIL = False  # rely on zero-initialized ExternalOutput buffers

import os
NORM_ENGINE = os.environ.get("AK_NORM", "dve")   # gpsimd | dve | skip
LOAD_PATH = os.environ.get("AK_LOADS", "swdge")     # swdge | synccast
SKIP_B = os.environ.get("AK_SKIP_B", "0") == "1"
SKIP_PRIOR = os.environ.get("AK_SKIP_PRIOR", "0") == "1"
SKIP_SERIES_DMA = os.environ.get("AK_SKIP_SDMA", "0") == "1"
TUNE = os.environ.get("AK_TUNE", "1") == "1"

_NC_CACHE = {}


def build_nc(repeat=1):
    """Build + compile the per-core Bass graph (identical on all 8 cores).

    repeat>1 wraps the computation in a hardware loop (identical work
    each iteration) — used only for on-device timing measurement.
    """
    key = (repeat, NORM_ENGINE, LOAD_PATH, SKIP_B, SKIP_PRIOR,
           SKIP_SERIES_DMA, USE_F32R, TUNE)
    if key in _NC_CACHE:
        return _NC_CACHE[key]

    import concourse.bass as bass
    import concourse.mybir as mybir
    import concourse.tile as tile
    from concourse import bacc
    from concourse.bass import ts
    from concourse.masks import make_identity

    f32 = mybir.dt.float32
    f32r = mybir.dt.float32r
    i32 = mybir.dt.int32
    Act = mybir.ActivationFunctionType
    Alu = mybir.AluOpType

    nc = bacc.Bacc("TRN2", target_bir_lowering=False, debug=False,
                   num_devices=NCORES)

    qT = nc.dram_tensor("qt", [PER, E, L], f32, kind="ExternalInput").ap()
    kT = nc.dram_tensor("kt", [PER, E, L], f32, kind="ExternalInput").ap()
    vI = nc.dram_tensor("v", [PER, L, E], f32, kind="ExternalInput").ap()
    sgI = nc.dram_tensor("sg", [PER, PB, NB], f32, kind="ExternalInput").ap()
    o_series = nc.dram_tensor("o_series", [PER, L, L], f32,
                              kind="ExternalOutput").ap()
    o_prior = nc.dram_tensor("o_prior", [PER, L, L], f32,
                             kind="ExternalOutput").ap()
    o_v = nc.dram_tensor("o_v", [PER, L, E], f32, kind="ExternalOutput").ap()
    o_sig = nc.dram_tensor("o_sig", [PER, PB, NB], f32,
                           kind="ExternalOutput").ap()

    with tile.TileContext(nc) as tc:
        with (
            tc.tile_pool(name="const", bufs=1) as constp,
            tc.tile_pool(name="initmp", bufs=1) as initmp,
            tc.tile_pool(name="qk", bufs=2) as qkp,
            tc.tile_pool(name="vin", bufs=2) as vinp,
            tc.tile_pool(name="sigp", bufs=2) as sigp,
            tc.tile_pool(name="pwork", bufs=7 if TUNE else 6) as pwork,
            tc.tile_pool(name="ptbp", bufs=4 if TUNE else 3) as ptbp,
            tc.tile_pool(name="priorp", bufs=4 if TUNE else 3) as priorp,
            tc.tile_pool(name="vtp", bufs=2) as vtp,
            tc.tile_pool(name="voutp", bufs=2) as voutp,
            tc.tile_pool(name="psA", bufs=2, space="PSUM") as psA,
            tc.tile_pool(name="psT", bufs=2, space="PSUM") as psT,
            tc.tile_pool(name="psV", bufs=1, space="PSUM") as psV,
            tc.tile_pool(name="psR", bufs=1, space="PSUM") as psR,
        ):
            # ---- static constants -------------------------------------
            # d2[p, j] = (2047 + p - j)^2  -> row-block l0 reads the window
            # j in [2047-l0, 4095-l0) so that d2 = (l0+p - s)^2.
            d2i = initmp.tile([PB, 2 * L], i32)
            nc.gpsimd.iota(d2i, pattern=[[-1, 2 * L]], base=L - 1,
                           channel_multiplier=1)
            d2f = initmp.tile([PB, 2 * L], f32)
            nc.vector.tensor_copy(d2f, d2i)
            d2 = constp.tile([PB, 2 * L], f32)
            nc.vector.tensor_tensor(d2, d2f, d2f, Alu.mult)

            # additive causal masks for the diagonal 512-chunk, by row-block
            # offset m = i % 4: keep (0) where f <= 128*m + p else NEG.
            maskA = []
            for m in range(4):
                mk = constp.tile([PB, CH], f32, tag=f"maskA{m}")
                nc.gpsimd.memset(mk, 0.0)
                nc.gpsimd.affine_select(
                    out=mk, in_=mk, compare_op=Alu.is_ge, fill=NEG,
                    base=PB * m, pattern=[[-1, CH]], channel_multiplier=1)
                maskA.append(mk)

            ident = constp.tile([PB, PB], f32)
            make_identity(nc, ident)

            bias3 = constp.tile([PB, 1], f32, tag="bias3")
            nc.vector.memset(bias3, LN3 * 1e-5)

            mmf = f32r if USE_F32R else f32
            mmdma = nc.gpsimd.dma_start if USE_F32R else nc.sync.dma_start

            def body():
                # ---- sigma transforms, grouped per ACT function so
                # walrus emits few ACT table switches --------------------
                sig_t1, sig_val, sig_a, sig_lnc = {}, {}, {}, {}
                for h in range(PER):
                    sg_raw = sigp.tile([PB, NB], f32, tag=f"sgraw{h}", bufs=1)
                    nc.sync.dma_start(sg_raw, sgI[h])
                    t1 = sigp.tile([PB, NB], f32, tag=f"t1{h}", bufs=1)
                    nc.scalar.activation(t1, sg_raw, Act.Sigmoid, scale=5.0)
                    sig_t1[h] = t1
                for h in range(PER):
                    # sig = 3^(sigmoid(5x) + 1e-5) - 1
                    sigval = sigp.tile([PB, NB], f32, tag=f"sigval{h}",
                                       bufs=1)
                    nc.scalar.activation(sigval, sig_t1[h], Act.Exp,
                                         bias=bias3[:, 0:1], scale=LN3)
                    nc.vector.tensor_scalar_add(sigval, sigval, -1.0)
                    nc.sync.dma_start(o_sig[h], sigval)
                    sig_val[h] = sigval
                for h in range(PER):
                    sigval = sig_val[h]
                    sq = sigp.tile([PB, NB], f32, tag=f"sq{h}", bufs=1)
                    nc.vector.tensor_tensor(sq, sigval, sigval, Alu.mult)
                    rq = sigp.tile([PB, NB], f32, tag=f"rq{h}", bufs=1)
                    nc.vector.reciprocal(rq, sq)
                    a_coef = sigp.tile([PB, NB], f32, tag=f"acoef{h}", bufs=1)
                    nc.vector.tensor_scalar_mul(a_coef, rq, -0.5)
                    sig_a[h] = a_coef
                    lg = sigp.tile([PB, NB], f32, tag=f"lg{h}", bufs=1)
                    nc.scalar.activation(lg, sigval, Act.Ln)
                    lnc = sigp.tile([PB, NB], f32, tag=f"lnc{h}", bufs=1)
                    nc.vector.tensor_scalar(lnc, lg, -1.0, -HALF_LN_2PI,
                                            Alu.mult, Alu.add)
                    sig_lnc[h] = lnc

                # ---- per-pair ("head") loop ----------------------------
                for h in range(PER):
                    if LOAD_PATH == "swdge" or not USE_F32R:
                        qt_sb = qkp.tile([E, L], mmf, tag="qt")
                        mmdma(qt_sb, qT[h])
                        kt_sb = qkp.tile([E, L], mmf, tag="kt")
                        mmdma(kt_sb, kT[h])
                        v_sb = vinp.tile([PB, NB * E], mmf, tag="v")
                        mmdma(v_sb, vI[h].rearrange("(n p) e -> p n e",
                                                    p=PB))
                    else:
                        qt_f = qkp.tile([E, L], f32, tag="qtf")
                        nc.sync.dma_start(qt_f, qT[h])
                        qt_sb = qkp.tile([E, L], mmf, tag="qt")
                        nc.vector.tensor_copy(qt_sb, qt_f)
                        kt_f = qkp.tile([E, L], f32, tag="ktf")
                        nc.sync.dma_start(kt_f, kT[h])
                        kt_sb = qkp.tile([E, L], mmf, tag="kt")
                        nc.vector.tensor_copy(kt_sb, kt_f)
                        v_f = vinp.tile([PB, NB * E], f32, tag="vf")
                        nc.sync.dma_start(
                            v_f, vI[h].rearrange("(n p) e -> p n e", p=PB))
                        v_sb = vinp.tile([PB, NB * E], mmf, tag="v")
                        nc.vector.tensor_copy(v_sb, v_f)

                    a_coef, lnc = sig_a[h], sig_lnc[h]
                    rs_parts = sigp.tile([PB, 2 * NB], f32, tag="rsparts")
                    rs_all = sigp.tile([PB, NB], f32, tag="rsall")
                    rinv_all = sigp.tile([PB, NB], f32, tag="rinvall")

                    p_tiles = [None] * NB
                    vout_sb = None

                    for i in range(NB):
                        l0 = i * PB

                        # ---- prior row-block: banded -------------------
                        c0 = max(0, l0 - BW)
                        c1 = min(L, l0 + PB + BW)
                        W = c1 - c0
                        pr = priorp.tile([PB, PB + 2 * BW], f32, tag="prior")
                        if SKIP_PRIOR:
                            pr = None
                        if not SKIP_PRIOR:
                            nc.scalar.activation(
                            pr[:, 0:W],
                            d2[:, L - 1 - l0 + c0:L - 1 - l0 + c0 + W],
                                Act.Exp, bias=lnc[:, i:i + 1],
                                scale=a_coef[:, i:i + 1])
                            nc.scalar.dma_start(
                                o_prior[h, ts(i, PB), c0:c1], pr[:, 0:W])

                        # ---- series row-block (phase A) ----------------
                        nch = i // 4 + 1
                        w = nch * CH
                        p_sb = pwork.tile([PB, L], f32, tag="p")
                        p_tiles[i] = p_sb
                        nhalf = (nch + 1) // 2
                        for g in range(nhalf):
                            ps = psA.tile([PB, 2 * CH], f32, tag="psa")
                            cw = min(2 * CH, w - g * 2 * CH)
                            for cc in range(g * 2, min(g * 2 + 2, nch)):
                                nc.tensor.matmul(
                                    ps[:, ts(cc - g * 2, CH)],
                                    qt_sb[:, ts(i, PB)],
                                    kt_sb[:, ts(cc, CH)],
                                    start=True, stop=True)
                            if g == nhalf - 1:
                                # mask the diag chunk's maskable columns
                                moff = (nch - 1) - g * 2
                                mb0 = PB * (i % 4)
                                nc.vector.tensor_tensor(
                                    ps[:, moff * CH + mb0:(moff + 1) * CH],
                                    ps[:, moff * CH + mb0:(moff + 1) * CH],
                                    maskA[i % 4][:, mb0:], Alu.add)
                            nc.scalar.activation(
                                p_sb[:, g * 2 * CH:g * 2 * CH + cw],
                                ps[:, 0:cw], Act.Exp, scale=SCALE,
                                accum_out=rs_parts[:,
                                                   2 * i + g:2 * i + g + 1])
                        if nhalf == 1:
                            nc.vector.reciprocal(
                                rinv_all[:, i:i + 1],
                                rs_parts[:, 2 * i:2 * i + 1])
                        else:
                            nc.vector.reduce_sum(
                                rs_all[:, i:i + 1],
                                rs_parts[:, 2 * i:2 * i + nhalf],
                                mybir.AxisListType.X)
                            nc.vector.reciprocal(rinv_all[:, i:i + 1],
                                                 rs_all[:, i:i + 1])
                        if NORM_ENGINE == "gpsimd":
                            nc.gpsimd.tensor_scalar_mul(
                                p_sb[:, 0:w], p_sb[:, 0:w],
                                rinv_all[:, i:i + 1])
                        elif NORM_ENGINE == "dve":
                            nc.vector.tensor_scalar_mul(
                                p_sb[:, 0:w], p_sb[:, 0:w],
                                rinv_all[:, i:i + 1])
                        if not SKIP_SERIES_DMA:
                            nc.sync.dma_start(o_series[h, ts(i, PB), 0:w],
                                              p_sb[:, 0:w])

                        # ---- phase B every 4 row-blocks ----------------
                        if SKIP_B or i % 4 != 3:
                            continue
                        j = i // 4
                        vt_ps = psV.tile([E, 4 * PB], f32, tag="vt")
                        for t in range(4 * j + 4):
                            m0 = max(0, t - 4 * j)
                            tr_ps = psT.tile([PB, 4 * PB], f32, tag="tr")
                            for m in range(m0, 4):
                                nc.tensor.transpose(
                                    tr_ps[:, ts(m, PB)],
                                    p_tiles[4 * j + m][:, ts(t, PB)],
                                    ident)
                            ptb = ptbp.tile([PB, 4 * PB], mmf, tag="ptb")
                            if TUNE and t % 2 == 1:
                                nc.scalar.copy(ptb[:, m0 * PB:],
                                               tr_ps[:, m0 * PB:])
                            else:
                                nc.vector.tensor_copy(ptb[:, m0 * PB:],
                                                      tr_ps[:, m0 * PB:])
                            nc.tensor.matmul(
                                vt_ps[:, m0 * PB:],
                                v_sb[:, ts(t, E)],
                                ptb[:, m0 * PB:],
                                start=(t == 0), stop=(t == 4 * j + 3))
                        vt_sb = vtp.tile([E, 4 * PB], f32, tag="vtsb")
                        nc.vector.tensor_copy(vt_sb, vt_ps)
                        tr2_ps = psR.tile([PB, 4 * E], f32, tag="tr2")
                        for m in range(4):
                            nc.tensor.transpose(
                                tr2_ps[:, ts(m, E)],
                                vt_sb[:, ts(m, PB)],
                                ident[0:E, 0:E])
                        if j == 0:
                            vout_sb = voutp.tile([PB, NB * E], f32,
                                                 tag="vout")
                        nc.vector.tensor_copy(vout_sb[:, ts(j, 4 * E)],
                                              tr2_ps)
                        if j == 3:
                            nc.sync.dma_start(
                                o_v[h].rearrange("(n p) e -> p n e", p=PB),
                                vout_sb)

            if repeat > 1:
                with tc.For_i(0, repeat, 1):
                    body()
            else:
                body()

    nc.compile()
    _NC_CACHE[key] = nc
    return nc


def _shard_inputs(queries, keys, values, sigma):
    q = np.asarray(queries, dtype=np.float32)
    k = np.asarray(keys, dtype=np.float32)
    v = np.asarray(values, dtype=np.float32)
    sg = np.asarray(sigma, dtype=np.float32)

    qT = np.ascontiguousarray(q.transpose(0, 2, 3, 1)).reshape(PAIRS, E, L)
    kT = np.ascontiguousarray(k.transpose(0, 2, 3, 1)).reshape(PAIRS, E, L)
    vA = np.ascontiguousarray(v.transpose(0, 2, 1, 3)).reshape(PAIRS, L, E)
    # sigma [B, L, H] -> [pairs, L] -> [pairs, 128, 16] with l = n*128 + p
    sgA = sg.transpose(0, 2, 1).reshape(PAIRS, NB, PB).transpose(0, 2, 1)
    sgA = np.ascontiguousarray(sgA)

    in_maps = []
    for c in range(NCORES):
        s = slice(c * PER, (c + 1) * PER)
        in_maps.append({
            "qt": np.ascontiguousarray(qT[s]),
            "kt": np.ascontiguousarray(kT[s]),
            "v": np.ascontiguousarray(vA[s]),
            "sg": np.ascontiguousarray(sgA[s]),
        })
    return in_maps


def _assemble_outputs(results):
    series = np.concatenate([r["o_series"] for r in results], axis=0)
    series = series.reshape(B, H, L, L)
    prior = np.concatenate([r["o_prior"] for r in results], axis=0)
    prior = prior.reshape(B, H, L, L)
    vout = np.concatenate([r["o_v"] for r in results], axis=0)
    vout = vout.reshape(B, H, L, E).transpose(0, 2, 1, 3)
    vout = np.ascontiguousarray(vout)
    sig_pn = np.concatenate([r["o_sig"] for r in results], axis=0)
    sig_bhl = sig_pn.transpose(0, 2, 1).reshape(B, H, L)
    sig_full = np.broadcast_to(sig_bhl[..., None], (B, H, L, L))
    return vout, series, prior, sig_full


def kernel(**inputs):
    from concourse.bass_utils import run_bass_kernel_spmd

    nc = build_nc()
    in_maps = _shard_inputs(inputs["queries"], inputs["keys"],
                            inputs["values"], inputs["sigma"])
    res = run_bass_kernel_spmd(nc, in_maps, core_ids=list(range(NCORES)),
                               trace=False)
    return _assemble_outputs(res.results)
